# revision 31
# baseline (speedup 1.0000x reference)
"""Trainium2 Bass kernel for nn_Attention_85074712199827.

Computes, for hidden [1,32,1024], encoder_outputs [32,2048,1024],
W_attn [1024,2048], b_attn [1024], v [1024]:

    h_proj  = hidden[0] @ W_attn[:, :1024].T
    e_proj  = encoder_outputs @ W_attn[:, 1024:].T
    energy  = tanh(e_proj + h_proj[:, None, :] + b_attn)
    att     = energy @ v
    out     = softmax(att, axis=1)          # [32, 2048] float32

Distribution: data-parallel over the batch across 8 NeuronCores (4
batch rows per core); the tiny parameters are replicated.  h_proj +
b_attn (67 MFLOP, 0.04% of the work) is precomputed on the host and
shipped as a [128, 8, 4] bias tensor, so the device only runs the
e_proj pipeline.

Per-core schedule: enc streams HBM->SBUF fp32 on the sync HWDGE ring
(loads only), DVE casts it to bf16 at the head of each iteration's DVE
stream (the loads landed an iteration earlier, so the casts never
block), and the scalar HWDGE ring xbar-transposes to [h, s] layout
with TWO units of lookahead -- each unit's 8 transposes have ~2
iterations of slack, so scalar-ring jitter no longer starves the PE.
The e_proj matmuls alternate PSUM banks (`for hc: for c0`); back-to-
back accumulation into the same bank costs ~50ns per matmul.  The
v-dot is NOT done with 1-row PE matmuls: the tanh output is scaled by
v on the scalar engine, the 8 o-chunks are summed on DVE, and a single
ones-vector matmul reduces the 128 partitions.  Softmax runs per batch
row as soon as that row's logits are complete.

Self-contained: only environment packages (concourse, numpy, ml_dtypes)
are imported; all shapes/sharding are hardcoded for this problem.
"""

from contextlib import ExitStack

import ml_dtypes
import numpy as np

import concourse.bass as bass
import concourse.tile as tile
from concourse import bacc, mybir

F32 = mybir.dt.float32
BF16 = mybir.dt.bfloat16
AF = mybir.ActivationFunctionType
ADD = mybir.AluOpType.add
P = 128


def build_nc(b_loc=4, s=2048, h=1024, n_cores=8, sb=1024,
             warmup_mm=13, startup_keep=4, keepalive_mm=0,
             first_sb=512, ld_chunk=512):
    n_hc = h // P           # contraction chunks
    n_ot = h // P           # output (o) tiles

    nc = bacc.Bacc("TRN2", target_bir_lowering=False, debug=False,
                   num_devices=n_cores)

    wt = nc.dram_tensor("wt", [h, h], BF16, kind="ExternalInput").ap()
    # cf32: [128, 40] = h_proj+bias [128, 8*4] | vT [128, 8]
    cf32 = nc.dram_tensor("cf32", [P, n_ot * b_loc + n_ot], F32,
                          kind="ExternalInput").ap()
    # cbf: [128, 513] = ones [128,1] | zeros [128,512]
    cbf = nc.dram_tensor("cbf", [P, 513], BF16, kind="ExternalInput").ap()
    enc = nc.dram_tensor("enc", [b_loc, s, h], F32, kind="ExternalInput").ap()
    out = nc.dram_tensor("out", [b_loc, s], F32, kind="ExternalOutput").ap()

    with tile.TileContext(nc) as tc, ExitStack() as ctx:
        const = ctx.enter_context(tc.tile_pool(name="const", bufs=1))
        psmall = ctx.enter_context(tc.tile_pool(name="psmall", bufs=1, space="PSUM"))

        # ---- PE warmup: dependency-free matmuls to lift the HAM clock
        # gate to 8/8 while the first enc block is still in flight ----
        cbf_t = const.tile([P, 513], BF16)
        nc.scalar.dma_start(cbf_t[:], cbf)
        ones_bf = cbf_t[:, 0:1]
        wz = cbf_t[:, 1:513]
        for i in range(warmup_mm):
            pw = psmall.tile([P, 512], F32, name="pw", tag="ps")
            nc.tensor.matmul(pw[:], wz[:, :P], wz[:], start=True, stop=True)

        def keepalive(n):
            for _ in range(n):
                pw = psmall.tile([P, 512], F32, name="pw", tag="ps")
                nc.tensor.matmul(pw[:], wz[:, :P], wz[:], start=True, stop=True)

        # ---- weights + packed constants; scalar-ring DMAs complete
        # serially (~2-3us each), so emission order = arrival order ----
        wt_bf = const.tile([P, n_hc, h], BF16)
        wt_r = wt.rearrange("(jc p) o -> p jc o", p=P)
        q = n_hc // 2

        def emit_w(c):
            nc.scalar.dma_start(
                wt_bf[:, c * q:(c + 1) * q, :],
                wt_r[:, c * q:(c + 1) * q, :])

        cf_t = const.tile([P, n_ot * b_loc + n_ot], F32)

        def emit_consts():
            nc.scalar.dma_start(cf_t[:], cf32)

        def hb_ap(ot, b):
            return cf_t[:, ot * b_loc + b:ot * b_loc + b + 1]

        def vt_ap(ot):
            return cf_t[:, n_ot * b_loc + ot:n_ot * b_loc + ot + 1]

        # ---- main pipeline pools ----
        inp = ctx.enter_context(tc.tile_pool(name="inp", bufs=3))
        bfp = ctx.enter_context(tc.tile_pool(name="bfp", bufs=4))
        encT_p = ctx.enter_context(tc.tile_pool(name="encT", bufs=3))
        en_p = ctx.enter_context(tc.tile_pool(name="energy", bufs=3))
        tmp_p = ctx.enter_context(tc.tile_pool(name="vtmp", bufs=3))
        acc_p = ctx.enter_context(tc.tile_pool(name="acc", bufs=3))
        row_p = ctx.enter_context(tc.tile_pool(name="rowbuf", bufs=1))
        pe_p = ctx.enter_context(tc.tile_pool(name="psum_e", bufs=2, space="PSUM"))
        pa_p = ctx.enter_context(tc.tile_pool(name="psum_att", bufs=2, space="PSUM"))

        # per-row logits buffers (DVE/ACT accesses must start at partition 0)
        att_rows = [const.tile([1, s], F32, name=f"attrow{b}")
                    for b in range(b_loc)]

        # units: (b, s0, size) — b-major so softmax(b) pipelines.
        # All of b=0 runs in small units so the pipeline ramps fast.
        units = []
        for b in range(b_loc):
            step = first_sb if b == 0 else sb
            for s0 in range(0, s, step):
                units.append((b, s0, step))

        def phase1a(unit):
            # HBM -> SBUF fp32 loads on the sync HWDGE ring, issued two
            # iterations before their casts so the casts never wait.
            b, s0, sz = unit
            chunk = P if (b == 0 and s0 == 0) else ld_chunk
            its = []
            for c0 in range(0, sz, chunk):
                csz = min(chunk, sz - c0)
                it = inp.tile([P, csz // P, h], F32, name="it")
                nc.sync.dma_start(
                    it[:], enc[b, s0 + c0:s0 + c0 + csz, :].rearrange(
                        "(si p) h -> p si h", p=P))
                its.append(it)
            return its

        def phase1b(its):
            # DVE casts fp32 -> bf16 at the HEAD of an iteration's DVE
            # stream; putting them last (behind the v-muls, which pace to
            # the iteration's end) made the transposes 10us+ late.
            bts = []
            for it in its:
                bt = bfp.tile([P, it.shape[1], h], BF16, name="bt")
                nc.vector.tensor_copy(out=bt[:], in_=it[:])
                bts.append(bt)
            return bts

        def phase2(unit, bts):
            # SBUF xbar transpose [s,h] bf16 -> [h,s] on the scalar HWDGE
            # ring.  Returns (eT, thunks): the transposes are emitted by
            # phase3_mm between ACTIVATEs (a block of 8 head-of-line-
            # blocks the ACTIVATEs -> PSUM backpressure stalls the PE).
            b, s0, sz = unit
            eT = encT_p.tile([P, n_hc, sz], BF16, name="eT")
            thunks = []
            col = 0
            for bt in bts:
                for si in range(bt.shape[1]):
                    def t(bt=bt, si=si, col=col):
                        nc.scalar.dma_start_transpose(
                            eT[:, :, col:col + P], bt[:, si, :])
                    thunks.append(t)
                    col += P
            return eT, thunks

        def phase3_mm(unit, eT, next_thunks=()):
            # PSUM matmul output must fit one 2KB bank -> 512-wide chunks.
            # Consecutive matmuls MUST alternate PSUM banks (same-bank
            # accumulation costs ~50ns/matmul), hence `for hc: for c0`.
            # tanh on ACT, x v_o and the ot-accumulation on DVE.
            b, s0, sz = unit
            acc = acc_p.tile([P, sz], BF16, name="acc")
            tq = list(next_thunks)
            for ot in range(n_ot):
                eng = en_p.tile([P, sz], BF16, name="eng")
                pe = pe_p.tile([P, sz], F32, name="pe")
                for hc in range(n_hc):
                    for c0 in range(0, sz, 512):
                        nc.tensor.matmul(
                            pe[:, c0:c0 + 512],
                            wt_bf[:, hc, ot * P:(ot + 1) * P],
                            eT[:, hc, c0:c0 + 512],
                            start=(hc == 0), stop=(hc == n_hc - 1))
                nc.scalar.activation(
                    eng[:], pe[:], AF.Tanh, bias=hb_ap(ot, b))
                for _ in range(2):
                    if tq:
                        tq.pop(0)()
                if ot == 0:
                    nc.vector.tensor_scalar_mul(acc[:], eng[:], vt_ap(0))
                else:
                    tmp = tmp_p.tile([P, sz], BF16, name="tmp")
                    nc.vector.tensor_scalar_mul(tmp[:], eng[:], vt_ap(ot))
                    nc.vector.tensor_tensor(acc[:], acc[:], tmp[:], ADD)
            for t in tq:
                t()
            return acc

        def phase3_fin(unit, acc):
            # partition-reduce via ones-matmul (emitted one unit late so
            # the PE never waits on the DVE acc), then DVE copies the
            # logits PSUM->SBUF.
            b, s0, sz = unit
            for c0 in range(0, sz, 512):
                pa = pa_p.tile([P, 512], F32, name="pa", tag="pa")
                nc.tensor.matmul(
                    pa[0:1, :], ones_bf, acc[:, c0:c0 + 512],
                    start=True, stop=True)
                nc.vector.tensor_copy(
                    out=att_rows[b][0:1, s0 + c0:s0 + c0 + 512],
                    in_=pa[0:1, :])

        def softmax_row(b):
            # |att| < ~6, so exp() is safe in fp32 without the row max.
            e_row = row_p.tile([1, s], F32, name="erow")
            ssum = const.tile([1, 1], F32, name=f"ssum{b}")
            nc.scalar.activation(
                e_row[:], att_rows[b][:], AF.Exp, accum_out=ssum[:])
            rinv = const.tile([1, 1], F32, name=f"rinv{b}")
            nc.vector.reciprocal(rinv[:], ssum[:])
            nc.vector.tensor_scalar_mul(e_row[:], e_row[:], rinv[:])
            nc.gpsimd.dma_start(out[b:b + 1, :], e_row[:])

        # ---- software pipeline, transposes TWO units ahead:
        #   iter i: casts(u_{i+2}) at the DVE head | matmuls(u_i) with
        #           transposes(u_{i+2}) interleaved | loads(u_{i+3}) |
        #           fin(u_{i-1}) | softmax(row completed at u_{i-2})
        n_u = len(units)
        its_q = {}
        eT_q = {}
        its_q[0] = phase1a(units[0])
        emit_w(0)
        emit_w(1)
        emit_consts()
        eT_q[0], th0 = phase2(units[0], phase1b(its_q.pop(0)))
        for t in th0:
            t()
        if 1 < n_u:
            its_q[1] = phase1a(units[1])
            eT_q[1], th1 = phase2(units[1], phase1b(its_q.pop(1)))
            for t in th1:
                t()
        if 2 < n_u:
            its_q[2] = phase1a(units[2])
        if 3 < n_u:
            its_q[3] = phase1a(units[3])

        fin = None
        sm_row = None
        for i, u in enumerate(units):
            thunks = ()
            if i + 2 < n_u:
                bts = phase1b(its_q.pop(i + 2))
                eT_q[i + 2], thunks = phase2(units[i + 2], bts)
            acc = phase3_mm(u, eT_q.pop(i), thunks)
            if i + 4 < n_u:
                its_q[i + 4] = phase1a(units[i + 4])
            if sm_row is not None:
                softmax_row(sm_row)
                sm_row = None
            if fin is not None:
                phase3_fin(*fin)
                fb, fs0, fsz = fin[0]
                if fs0 + fsz == s:
                    sm_row = fb
            fin = (u, acc)
            if i == 0:
                keepalive(startup_keep)
            else:
                keepalive(keepalive_mm)
        phase3_fin(*fin)
        softmax_row(b_loc - 1)

    nc.compile()
    return nc


def make_in_maps(hidden, encoder_outputs, W_attn, b_attn, v, n_cores=8):
    hidden = np.asarray(hidden, dtype=np.float32)
    encoder_outputs = np.asarray(encoder_outputs, dtype=np.float32)
    W_attn = np.asarray(W_attn, dtype=np.float32)
    b_attn = np.asarray(b_attn, dtype=np.float32)
    v = np.asarray(v, dtype=np.float32)

    b = encoder_outputs.shape[0]
    h = W_attn.shape[0]
    n_ot = h // P
    b_loc = b // n_cores
    # device only needs We.T; h_proj + b_attn precomputed here (67 MFLOP)
    wt = np.ascontiguousarray(W_attn[:, h:].T.astype(ml_dtypes.bfloat16))
    hp = hidden[0] @ W_attn[:, :h].T + b_attn          # [B, h] fp32
    vT = v.reshape(n_ot, P).T                          # [128, 8]
    cbf = np.zeros((P, 513), dtype=ml_dtypes.bfloat16)
    cbf[:, 0] = 1.0
    in_maps = []
    for i in range(n_cores):
        bsl = slice(b_loc * i, b_loc * (i + 1))
        # hbias[p, oc, b] = hp[b, oc*128 + p]
        hbias = hp[bsl].T.reshape(n_ot, P, b_loc).transpose(1, 0, 2)
        cf32 = np.concatenate(
            [hbias.reshape(P, n_ot * b_loc), vT], axis=1).astype(np.float32)
        in_maps.append({
            "wt": wt,
            "cf32": np.ascontiguousarray(cf32),
            "cbf": cbf,
            "enc": np.ascontiguousarray(encoder_outputs[bsl]),
        })
    return in_maps


_NC_CACHE = {}


def _get_nc():
    if "nc" not in _NC_CACHE:
        _NC_CACHE["nc"] = build_nc(b_loc=4, s=2048, h=1024, n_cores=8)
    return _NC_CACHE["nc"]


def kernel(hidden, encoder_outputs, W_attn, b_attn, v):
    from concourse.bass_utils import run_bass_kernel_spmd

    nc = _get_nc()
    in_maps = make_in_maps(hidden, encoder_outputs, W_attn, b_attn, v,
                           n_cores=8)
    res = run_bass_kernel_spmd(nc, in_maps, core_ids=list(range(8)))
    out = np.concatenate([np.asarray(res.results[i]["out"])
                          for i in range(8)], axis=0)
    return out.astype(np.float32)


# revision 32
# speedup vs baseline: 1.1829x; 1.1829x over previous
"""Trainium2 Bass kernel for nn_Attention_85074712199827.

Computes, for hidden [1,32,1024], encoder_outputs [32,2048,1024],
W_attn [1024,2048], b_attn [1024], v [1024]:

    h_proj  = hidden[0] @ W_attn[:, :1024].T
    e_proj  = encoder_outputs @ W_attn[:, 1024:].T
    energy  = tanh(e_proj + h_proj[:, None, :] + b_attn)
    att     = energy @ v
    out     = softmax(att, axis=1)          # [32, 2048] float32

Distribution: data-parallel over the batch across 8 NeuronCores (4
batch rows per core); the tiny parameters are replicated.  h_proj +
b_attn (67 MFLOP, 0.04% of the work) is precomputed on the host and
shipped as a [128, 8, 4] bias tensor packed with vT into one constant
load, so the device only runs the e_proj pipeline.

Per-core schedule: enc streams HBM->SBUF fp32 (sync HWDGE queue), is
cast to bf16 (DVE), xbar-transposed to [h, s] layout (sync queue), and
the PE runs the e_proj matmuls with 512-wide moving operands.  The
matmuls of two ot-chunks are INTERLEAVED so consecutive matmuls always
accumulate into different PSUM banks -- back-to-back accumulation into
the same bank costs ~50ns per matmul (264ns vs 213ns measured).  The
v-dot is NOT done with 1-row PE matmuls: the tanh output is scaled by
v on the scalar engine (Copy activation with per-partition scale), the
8 o-chunks are summed on DVE, and a single ones-vector matmul reduces
the 128 partitions.  Softmax runs per batch row as soon as that row's
attention logits are complete, overlapped with the next row's matmuls.

Self-contained: only environment packages (concourse, numpy, ml_dtypes)
are imported; all shapes/sharding are hardcoded for this problem.
"""

from contextlib import ExitStack

import ml_dtypes
import numpy as np

import concourse.bass as bass
import concourse.tile as tile
from concourse import bacc, mybir

F32 = mybir.dt.float32
BF16 = mybir.dt.bfloat16
AF = mybir.ActivationFunctionType
ADD = mybir.AluOpType.add
P = 128


def build_nc(b_loc=4, s=2048, h=1024, n_cores=8, sb=512,
             warmup_mm=22, startup_keep=12, keepalive_mm=0,
             first_sb=512, ld_chunk=512):
    n_hc = h // P           # contraction chunks
    n_ot = h // P           # output (o) tiles

    nc = bacc.Bacc("TRN2", target_bir_lowering=False, debug=False,
                   num_devices=n_cores)

    wt = nc.dram_tensor("wt", [h, h], BF16, kind="ExternalInput").ap()
    # cf32: [128, 40] = h_proj+bias [128, 8*4] | vT [128, 8]
    cf32 = nc.dram_tensor("cf32", [P, n_ot * b_loc + n_ot], F32,
                          kind="ExternalInput").ap()
    # cbf: [128, 513] = ones [128,1] | zeros [128,512]
    cbf = nc.dram_tensor("cbf", [P, 513], BF16, kind="ExternalInput").ap()
    enc = nc.dram_tensor("enc", [b_loc, s, h], F32, kind="ExternalInput").ap()
    out = nc.dram_tensor("out", [b_loc, s], F32, kind="ExternalOutput").ap()

    with tile.TileContext(nc) as tc, ExitStack() as ctx:
        const = ctx.enter_context(tc.tile_pool(name="const", bufs=1))
        psmall = ctx.enter_context(tc.tile_pool(name="psmall", bufs=1, space="PSUM"))

        # ---- PE warmup: dependency-free matmuls to lift the HAM clock
        # gate to 8/8 while the first enc block is still in flight ----
        cbf_t = const.tile([P, 513], BF16)
        nc.scalar.dma_start(cbf_t[:], cbf)
        ones_bf = cbf_t[:, 0:1]
        wz = cbf_t[:, 1:513]
        for i in range(warmup_mm):
            pw = psmall.tile([P, 512], F32, name="pw", tag="ps")
            nc.tensor.matmul(pw[:], wz[:, :P], wz[:], start=True, stop=True)

        def keepalive(n):
            for _ in range(n):
                pw = psmall.tile([P, 512], F32, name="pw", tag="ps")
                nc.tensor.matmul(pw[:], wz[:, :P], wz[:], start=True, stop=True)

        # ---- weights (We.T only) + packed constants on the scalar
        # queue; off the sync queue that feeds the main pipeline ----
        wt_bf = const.tile([P, n_hc, h], BF16)
        wt_r = wt.rearrange("(jc p) o -> p jc o", p=P)
        q = n_hc // 2

        def emit_w(c):
            nc.scalar.dma_start(
                wt_bf[:, c * q:(c + 1) * q, :],
                wt_r[:, c * q:(c + 1) * q, :])

        emit_w(0)
        emit_w(1)

        cf_t = const.tile([P, n_ot * b_loc + n_ot], F32)
        nc.scalar.dma_start(cf_t[:], cf32)

        def hb_ap(ot, b):
            return cf_t[:, ot * b_loc + b:ot * b_loc + b + 1]

        def vt_ap(ot):
            return cf_t[:, n_ot * b_loc + ot:n_ot * b_loc + ot + 1]

        # ---- main pipeline pools ----
        inp = ctx.enter_context(tc.tile_pool(name="inp", bufs=3))
        bfp = ctx.enter_context(tc.tile_pool(name="bfp", bufs=3))
        encT_p = ctx.enter_context(tc.tile_pool(name="encT", bufs=3))
        en_p = ctx.enter_context(tc.tile_pool(name="energy", bufs=4))
        tmp_p = ctx.enter_context(tc.tile_pool(name="vtmp", bufs=4))
        acc_p = ctx.enter_context(tc.tile_pool(name="acc", bufs=3))
        row_p = ctx.enter_context(tc.tile_pool(name="rowbuf", bufs=2))
        pe_p = ctx.enter_context(tc.tile_pool(name="psum_e", bufs=4, space="PSUM"))
        pa_p = ctx.enter_context(tc.tile_pool(name="psum_att", bufs=2, space="PSUM"))

        att_rows = [const.tile([1, s], F32, name=f"attrow{b}")
                    for b in range(b_loc)]

        # units: (b, s0, size) — b-major so softmax(b) pipelines.
        units = []
        for b in range(b_loc):
            if b == 0 and first_sb < sb:
                for s0 in range(0, s, first_sb):
                    units.append((b, s0, first_sb))
            else:
                for s0 in range(0, s, sb):
                    units.append((b, s0, sb))

        def phase1(unit):
            # HBM -> SBUF fp32 loads on the sync HWDGE queue (SWDGE issue
            # on gpsimd measured 10-40us/load), then DVE casts to bf16.
            b, s0, sz = unit
            its = []
            for c0 in range(0, sz, ld_chunk):
                csz = min(ld_chunk, sz - c0)
                it = inp.tile([P, csz // P, h], F32, name="it")
                nc.sync.dma_start(
                    it[:], enc[b, s0 + c0:s0 + c0 + csz, :].rearrange(
                        "(si p) h -> p si h", p=P))
                its.append(it)
            bts = []
            for it in its:
                bt = bfp.tile([P, it.shape[1], h], BF16, name="bt")
                nc.vector.tensor_copy(out=bt[:], in_=it[:])
                bts.append(bt)
            return bts

        def phase2(unit, bts):
            # SBUF xbar transpose [s,h] bf16 -> [h,s] on the sync queue,
            # ahead of the next lookahead load in FIFO order.
            b, s0, sz = unit
            eT = encT_p.tile([P, n_hc, sz], BF16, name="eT")
            col = 0
            for bt in bts:
                for si in range(bt.shape[1]):
                    nc.sync.dma_start_transpose(
                        eT[:, :, col:col + P], bt[:, si, :])
                    col += P
            return eT

        def phase3_mm(unit, eT):
            # PSUM matmul output must fit one 2KB bank -> 512-wide
            # chunks.  The matmuls of two ot-chunks are interleaved so
            # consecutive matmuls hit different PSUM banks (back-to-back
            # accumulation into one bank costs ~50ns per matmul).
            # tanh on ACT, x v_o and the ot-accumulation on DVE.
            b, s0, sz = unit
            acc = acc_p.tile([P, sz], BF16, name="acc")
            for op in range(n_ot // 2):
                pes = [pe_p.tile([P, sz], F32, name="pe") for _ in range(2)]
                for hc in range(n_hc):
                    for k in range(2):
                        ot = 2 * op + k
                        nc.tensor.matmul(
                            pes[k][:],
                            wt_bf[:, hc, ot * P:(ot + 1) * P],
                            eT[:, hc, :],
                            start=(hc == 0), stop=(hc == n_hc - 1))
                for k in range(2):
                    ot = 2 * op + k
                    eng = en_p.tile([P, sz], BF16, name="eng")
                    nc.scalar.activation(
                        eng[:], pes[k][:], AF.Tanh, bias=hb_ap(ot, b))
                    if ot == 0:
                        nc.vector.tensor_scalar_mul(acc[:], eng[:], vt_ap(0))
                    else:
                        tmp = tmp_p.tile([P, sz], BF16, name="tmp")
                        nc.vector.tensor_scalar_mul(tmp[:], eng[:], vt_ap(ot))
                        nc.vector.tensor_tensor(acc[:], acc[:], tmp[:], ADD)
            return acc

        def phase3_fin(unit, acc):
            # partition-reduce via ones-matmul (emitted one unit late so
            # the PE never waits on the DVE acc), then DVE copies the
            # logits PSUM->SBUF.  This copy is the only op that waits on
            # the late ones-matmul, and it sits LAST in the DVE FIFO for
            # this iteration, so nothing upstream ever blocks behind it.
            b, s0, sz = unit
            for c0 in range(0, sz, 512):
                pa = pa_p.tile([P, 512], F32, name="pa", tag="pa")
                nc.tensor.matmul(
                    pa[0:1, :], ones_bf, acc[:, c0:c0 + 512],
                    start=True, stop=True)
                nc.vector.tensor_copy(
                    out=att_rows[b][0:1, s0 + c0:s0 + c0 + 512],
                    in_=pa[0:1, :])

        def softmax_row(b):
            # Runs 2+ units after row b's logits landed in SBUF, so every
            # input is long ready when each queue reaches these ops.
            # |att| < ~6, so exp() is safe in fp32 without the row max.
            e_row = row_p.tile([1, s], F32, name="erow")
            ssum = const.tile([1, 1], F32, name=f"ssum{b}")
            nc.scalar.activation(
                e_row[:], att_rows[b][:], AF.Exp, accum_out=ssum[:])
            rinv = const.tile([1, 1], F32, name=f"rinv{b}")
            nc.vector.reciprocal(rinv[:], ssum[:])
            nc.vector.tensor_scalar_mul(e_row[:], e_row[:], rinv[:])
            nc.gpsimd.dma_start(out[b:b + 1, :], e_row[:])

        # ---- software pipeline, 3-deep load lookahead:
        #   iter i: matmuls(u_i) | transposes(u_{i+1}) | loads+casts(u_{i+3})
        #           | fin(u_{i-1}) | softmax(row done at u_{i-2})
        LOOK = 3
        bts_q = {}
        bts_q[0] = phase1(units[0])
        eT_cur = phase2(units[0], bts_q.pop(0))
        for k in range(1, min(LOOK, len(units))):
            bts_q[k] = phase1(units[k])

        fin = None
        sm_row = None
        for i, u in enumerate(units):
            acc = phase3_mm(u, eT_cur)
            if i + 1 < len(units):
                eT_cur = phase2(units[i + 1], bts_q.pop(i + 1))
                if i + LOOK < len(units):
                    bts_q[i + LOOK] = phase1(units[i + LOOK])
            if sm_row is not None:
                softmax_row(sm_row)
                sm_row = None
            if fin is not None:
                phase3_fin(*fin)
                fb, fs0, fsz = fin[0]
                if fs0 + fsz == s:
                    sm_row = fb
            fin = (u, acc)
            if i == 0:
                keepalive(startup_keep)
            else:
                keepalive(keepalive_mm)
        phase3_fin(*fin)
        softmax_row(b_loc - 1)

    nc.compile()
    return nc


def make_in_maps(hidden, encoder_outputs, W_attn, b_attn, v, n_cores=8):
    hidden = np.asarray(hidden, dtype=np.float32)
    encoder_outputs = np.asarray(encoder_outputs, dtype=np.float32)
    W_attn = np.asarray(W_attn, dtype=np.float32)
    b_attn = np.asarray(b_attn, dtype=np.float32)
    v = np.asarray(v, dtype=np.float32)

    b = encoder_outputs.shape[0]
    h = W_attn.shape[0]
    n_ot = h // P
    b_loc = b // n_cores
    # device only needs We.T; h_proj + b_attn precomputed here (67 MFLOP)
    wt = np.ascontiguousarray(W_attn[:, h:].T.astype(ml_dtypes.bfloat16))
    hp = hidden[0] @ W_attn[:, :h].T + b_attn          # [B, h] fp32
    vT = v.reshape(n_ot, P).T                          # [128, 8]
    cbf = np.zeros((P, 513), dtype=ml_dtypes.bfloat16)
    cbf[:, 0] = 1.0
    in_maps = []
    for i in range(n_cores):
        bsl = slice(b_loc * i, b_loc * (i + 1))
        # hbias[p, oc, b] = hp[b, oc*128 + p]
        hbias = hp[bsl].T.reshape(n_ot, P, b_loc).transpose(1, 0, 2)
        cf32 = np.concatenate(
            [hbias.reshape(P, n_ot * b_loc), vT], axis=1).astype(np.float32)
        in_maps.append({
            "wt": wt,
            "cf32": np.ascontiguousarray(cf32),
            "cbf": cbf,
            "enc": np.ascontiguousarray(encoder_outputs[bsl]),
        })
    return in_maps


_NC_CACHE = {}


def _get_nc():
    if "nc" not in _NC_CACHE:
        _NC_CACHE["nc"] = build_nc(b_loc=4, s=2048, h=1024, n_cores=8)
    return _NC_CACHE["nc"]


def kernel(hidden, encoder_outputs, W_attn, b_attn, v):
    from concourse.bass_utils import run_bass_kernel_spmd

    nc = _get_nc()
    in_maps = make_in_maps(hidden, encoder_outputs, W_attn, b_attn, v,
                           n_cores=8)
    res = run_bass_kernel_spmd(nc, in_maps, core_ids=list(range(8)))
    out = np.concatenate([np.asarray(res.results[i]["out"])
                          for i in range(8)], axis=0)
    return out.astype(np.float32)
